# revision 1
# baseline (speedup 1.0000x reference)
"""Trainium2 Bass kernel for nn_Baseline_mb_24189255811183 (gnn_message_passing).

Full on-device SPMD implementation on 8 NeuronCores.

Sharding: paths are sharded 8-ways (2048 paths/core) per the sharding hint;
link_state [L,64] and device_state [N,64] are replicated on every core; the
per-link/per-node path-state segment reductions are computed as local partial
sums and AllReduce'd each message-passing iteration; parameters replicated.

Everything lives transposed in SBUF (features on partitions):
  - pssT [64, 9*2048]  path-state sequence table (slot = t*2048 + p).
  - x gathers (link_state[link_to_path], device_state[node_to_path]) and the
    segment-sum gathers (pss[pl0,pl1], pss[pn0,pn1]) run on the GPSIMD engine
    via ap_gather (SBUF free-dim gather; indices int16, wrapped 16-partition
    layout, static across iterations, precomputed on host).
  - Each GRU step stacks its two matmul inputs in one [128, 2048] tile
    (parts 0:64 = x, 64:128 = h) so one stationary [128,128] matrix computes
    x@wx_zr + h@wh_zr (gates) and a block-diagonal one computes (xc | hc).
    Gate biases ride the ACT ops' per-partition bias vectors.
  - The x-gather uses one merged [64, L+N] link|device table per step;
    matmuls chunk the free dim at 512 (one PSUM bank of f32).
  - The last iteration's link/device GRU updates are dead code and skipped.
  - Readout (relu MLP -> softplus -> capacity-weighted hop sum) fused on
    device; per-core output is the [2048] delay shard.

Host does only the cheap O(P*D) encoders and index packing (~0.05 s).

Launch-path optimizations (the axon relay moves inputs at ~100 MB/s and
charges one round trip per serial fetch, so bytes and tensor count matter):
  - inputs packed into 5 tensors/core (~0.9 MB): bf16 initial path state,
    1/8 shards of the link|device table and of the weight pack (both
    AllGathered on device; the weight pack shards by rows so the gather's
    flat concat reproduces the [128, 832] layout exactly), one int16 index
    pack (16-row wrap, replicated to 64 on device).
  - a local bass_exec/partition-id primitive pair with a pre-baked
    backend_config, so the warm path never imports concourse (saves the
    0.3 s import and the per-process zstd/b64 of the BIR); the jitted
    executable is cached in-process and the 8 output shards are fetched
    with overlapped async copies.
  - /tmp program cache (backend_config keyed on the builder source hash)
    skips the Bass re-trace, and the JAX persistent compilation cache skips
    the NEFF compile, in warm fresh processes. The compile hook (and
    concourse) is imported only when a marker shows a compile is needed;
    a failed compile falls back to installing the hook and retrying.
    Every cache layer degrades gracefully to a full build.
"""
import sys
sys.path.insert(0, '/opt/trn_rl_repo')
import numpy as np

P, T, L, K, N, K2, M, D = 16384, 8, 4096, 16, 2048, 32, 8, 64
ITER = 8
NCORES = 8
PLOC = P // NCORES              # 2048 paths per core
NSLOT = (T + 1) * PLOC          # 18432 pss slots (slot = t*PLOC + p)
ZSLOT = NSLOT                   # zero row for non-local segment entries
NELEM = NSLOT + 16              # padded ap_gather table size

_NC_CACHE = {}


# ---------------------------------------------------------------- host math
def _relu(v):
    return np.maximum(v, 0.0)


def _mlp2(x, w1, b1, w2, b2):
    return _relu(_relu(x @ w1 + b1) @ w2 + b2)


def _wrap_idx(idx_list):
    """int16 index list -> [64, n/16] wrapped layout for ap_gather."""
    n = idx_list.shape[0]
    w = idx_list.reshape(n // 16, 16).T.astype(np.int16)   # [16, n/16]
    return np.tile(w, (4, 1))                              # [64, n/16]


def _gru_mats(wx, bx, wh, bh):
    """Stationary matrices + bias vectors for the stacked-input GRU."""
    wx = np.asarray(wx, np.float32); wh = np.asarray(wh, np.float32)
    bx = np.asarray(bx, np.float32); bh = np.asarray(bh, np.float32)
    s2 = np.ascontiguousarray(np.vstack([wx[:, 0:128], wh[:, 0:128]]))  # [128,128]
    s3 = np.zeros((128, 128), np.float32)
    s3[0:64, 0:64] = wx[:, 128:192]
    s3[64:128, 64:128] = wh[:, 128:192]
    bzr = (bx + bh)[0:128].reshape(128, 1).astype(np.float32)
    bc = np.stack([bh[128:192], bx[128:192]], axis=1).astype(np.float32)  # [64,2]
    return s2, s3, bzr, bc


def _host_prep(inp):
    f = lambda k: np.ascontiguousarray(np.asarray(inp[k], np.float32))
    ft, fp, fps, cap = f('flow_traffic'), f('flow_packets'), f('flow_packet_size'), f('link_capacity')
    ltp, ntp = np.asarray(inp['link_to_path']), np.asarray(inp['node_to_path'])
    ptl, ptn, ltn = np.asarray(inp['path_to_link']), np.asarray(inp['path_to_node']), np.asarray(inp['link_to_node'])

    ldt = (np.asarray(inp['link_device_type']) == 0).astype(np.float32)[:, None]
    load = ft[ptl[:, :, 0], 0].sum(1)[:, None] / (cap * 1e9)
    path_state = _mlp2(np.concatenate([ft * 1e-4, fp * 1e-3, fps * 1e-3], 1),
                       f('pe_w1'), f('pe_b1'), f('pe_w2'), f('pe_b2'))
    link_state = _mlp2(np.concatenate([cap * 1e-2, load, ldt], 1),
                       f('le_w1'), f('le_b1'), f('le_w2'), f('le_b2'))
    dlm = link_state[ltn].sum(1).mean(1, keepdims=True)
    dev_enc = (np.asarray(inp['nodes']) == 0).astype(np.float32)[:, None]
    device_state = _mlp2(np.concatenate([dev_enc, dlm], 1),
                         f('de_w1'), f('de_b1'), f('de_w2'), f('de_b2'))

    s2p, s3p, bzrp, bcp = _gru_mats(inp['pgru_wx'], inp['pgru_bx'], inp['pgru_wh'], inp['pgru_bh'])
    s2l, s3l, bzrl, bcl = _gru_mats(inp['lgru_wx'], inp['lgru_bx'], inp['lgru_wh'], inp['lgru_bh'])
    s2d, s3d, bzrd, bcd = _gru_mats(inp['dgru_wx'], inp['dgru_bx'], inp['dgru_wh'], inp['dgru_bh'])
    rw1, rw2, rw3 = f('ro_w1'), f('ro_w2'), f('ro_w3')
    rb1 = f('ro_b1').reshape(32, 1); rb2 = f('ro_b2').reshape(16, 1)
    rb3 = f('ro_b3').reshape(1, 1)

    lnT = np.ascontiguousarray(
        np.concatenate([link_state.T, device_state.T], axis=1))  # [64, 6144]
    # one [128, 832] pack of every replicated parameter tensor
    wpack = np.zeros((128, 832), np.float32)
    for i, s in enumerate((s2p, s3p, s2l, s3l, s2d, s3d)):
        wpack[:, i * 128:(i + 1) * 128] = s
    wpack[:, 768:769] = bzrp; wpack[:, 769:770] = bzrl; wpack[:, 770:771] = bzrd
    wpack[0:64, 771:773] = bcp; wpack[0:64, 773:775] = bcl; wpack[0:64, 775:777] = bcd
    wpack[0:64, 777:809] = rw1; wpack[0:32, 809:825] = rw2; wpack[0:16, 825:826] = rw3
    wpack[0:32, 826:827] = rb1; wpack[0:16, 827:828] = rb2; wpack[0:1, 828:829] = rb3

    import ml_dtypes
    pl0, pl1 = ptl[:, :, 0].astype(np.int32), ptl[:, :, 1].astype(np.int32)
    pn0, pn1 = ptn[:, :, 0].astype(np.int32), ptn[:, :, 1].astype(np.int32)
    glb = pl1 * PLOC + pl0          # local idx before rebase, [L, K]
    gnb = pn1 * PLOC + pn0          # [N, K2]
    icap_all = (1.0 / cap[ltp, 0]).astype(np.float32)      # [P, T]

    in_maps = []
    for c in range(NCORES):
        lo = c * PLOC
        sl = slice(lo, lo + PLOC)
        ps0 = path_state[sl].T.astype(ml_dtypes.bfloat16)
        # merged x gather indices: per step t, [link idxs | node idxs + L]
        ixc = np.concatenate([ltp[sl].T.astype(np.int32),
                              ntp[sl].T + np.int32(L)], axis=1)  # [T, 2*PLOC]
        # segment-sum indices over the local pss table
        gl = np.where((pl0 >= lo) & (pl0 < lo + PLOC), glb - lo, ZSLOT)
        gn = np.where((pn0 >= lo) & (pn0 < lo + PLOC), gnb - lo, ZSLOT)
        # one [16, 10240] idx pack, single 16-row wrap (device replicates x4)
        ip = np.concatenate([ixc.reshape(-1), gl.reshape(-1), gn.reshape(-1)])
        ipack = ip.reshape(-1, 16).T.astype(np.int16)      # [16, 10240]
        icap = np.ascontiguousarray(icap_all[sl].T).reshape(1, T * PLOC)
        lnts = lnT[:, c * 768:(c + 1) * 768]
        wps = wpack[16 * c:16 * (c + 1), :]
        in_maps.append(dict(ps0=ps0, lnts=lnts, wps=wps,
                            ipack=ipack, icap=icap))
    return in_maps


# ------------------------------------------------------------- device kernel
def _build_nc():
    import concourse.bacc as bacc
    import concourse.tile as tile
    import concourse.mybir as mybir

    f32, bf16, i16 = mybir.dt.float32, mybir.dt.bfloat16, mybir.dt.int16
    AF = mybir.ActivationFunctionType
    ALU = mybir.AluOpType
    AX = mybir.AxisListType

    nc = bacc.Bacc("TRN2", target_bir_lowering=False, debug=False,
                   num_devices=NCORES)
    dr = {}
    for name, shape, dt in (
        ('ps0', [64, PLOC], bf16), ('lnts', [64, (L + N) // 8], f32),
        ('wps', [16, 832], f32),
        ('ipack', [16, (2 * T * PLOC + L * K + N * K2) // 16], i16),
        ('icap', [1, T * PLOC], f32),
    ):
        dr[name] = nc.dram_tensor(name, shape, dt, kind="ExternalInput").ap()
    out_d = nc.dram_tensor("out", [1, PLOC], f32, kind="ExternalOutput").ap()

    NB = 512     # matmul free-dim chunk (one PSUM bank of f32)
    NCH = PLOC // NB

    with tile.TileContext(nc) as tc:
        with (
            tc.tile_pool(name="persist", bufs=1) as P0,
            tc.tile_pool(name="psumA", bufs=1, space="PSUM") as PA,
            tc.tile_pool(name="psumB", bufs=1, space="PSUM") as PB,
            tc.tile_pool(name="dram", bufs=1, space="DRAM") as DR,
        ):
            pssT = P0.tile([64, NELEM], f32)
            pb0 = P0.tile([64, PLOC], bf16)
            nc.sync.dma_start(pb0[:], dr['ps0'])
            nc.vector.tensor_copy(pssT[:, 0:PLOC], pb0[:])
            nc.vector.memset(pssT[:, NSLOT:NELEM], 0.0)

            def gru_step(s2, s3, bzr, bc, xh, hprev0, hout, zg, rg, u, v, scr):
                """GRU update, stacked layout: xh [128, W] = (x | h).

                hprev0 [64, W] base-partition-0 copy of h; hout [64, W];
                zg, rg [64, W] bf16; u, v, scr [64, W] f32.
                """
                ta = PA.tile([128, PLOC], f32, tag="pa")   # (z~ | r~)
                tb = PB.tile([128, PLOC], f32, tag="pb")   # (xc | hc)
                for j in range(NCH):
                    s = slice(j * NB, (j + 1) * NB)
                    nc.tensor.matmul(ta[:, s], s2[:], xh[:, s], start=True, stop=True)
                    nc.tensor.matmul(tb[:, s], s3[:], xh[:, s], start=True, stop=True)
                nc.scalar.activation(zg, ta[0:64, :], AF.Sigmoid, bias=bzr[0:64, :])
                nc.scalar.activation(rg, ta[64:128, :], AF.Sigmoid, bias=bzr[64:128, :])
                nc.scalar.activation(u, tb[64:128, :], AF.Identity, bias=bc[:, 0:1])
                nc.vector.tensor_tensor(out=v, in0=rg, in1=u, op=ALU.mult)
                nc.vector.tensor_tensor(out=u, in0=tb[0:64, :], in1=v, op=ALU.add)
                nc.scalar.activation(v, u, AF.Tanh, bias=bc[:, 1:2])   # v = c
                nc.vector.tensor_tensor(out=u, in0=hprev0, in1=v, op=ALU.subtract)
                nc.vector.tensor_tensor(out=scr, in0=zg, in1=u, op=ALU.mult)
                nc.vector.tensor_tensor(out=hout, in0=v, in1=scr, op=ALU.add)

            with tc.tile_pool(name="iter", bufs=1) as P1:
                lnT = P1.tile([64, L + N], f32)     # link | device states
                slp = P1.tile([64, L + N], f32)     # partial SL | SN
                xh = P1.tile([128, PLOC], f32)      # (x | h) stacked
                nb1 = P1.tile([64, PLOC], f32)
                segb = P1.tile([64, 4 * PLOC], f32)
                u_ = segb[:, 0:PLOC]
                v_ = segb[:, PLOC:2 * PLOC]
                gx_ = segb[:, 2 * PLOC:4 * PLOC]    # merged x-gather landing
                zg_ = P1.tile([64, PLOC], bf16)
                rg_ = P1.tile([64, PLOC], bf16)
                NIP = (2 * T * PLOC + L * K + N * K2) // 16   # 10240
                ipk = P1.tile([64, NIP], i16)
                for q in range(4):
                    nc.sync.dma_start(ipk[16 * q:16 * (q + 1), :], dr['ipack'])
                # all-gather the weight pack from 16-row shards (row concat)
                wpb_i = DR.tile([16, 832], f32)
                wg = DR.tile([128, 832], f32)
                nc.gpsimd.dma_start(wpb_i[:], dr['wps'])
                nc.gpsimd.collective_compute(
                    "AllGather", ALU.bypass,
                    replica_groups=[list(range(NCORES))],
                    ins=[wpb_i.opt()], outs=[wg.opt()])
                WOF = {'s2p': 0, 's3p': 128, 's2l': 256, 's3l': 384,
                       's2d': 512, 's3d': 640}
                gwt = {}
                for nm, off in WOF.items():
                    gwt[nm] = P1.tile([128, 128], f32, name="t_" + nm)
                    nc.sync.dma_start(gwt[nm][:], wg[:, off:off + 128])
                for i, nm in enumerate(('bzrp', 'bzrl', 'bzrd')):
                    gwt[nm] = P1.tile([128, 1], f32, name="t_" + nm)
                    nc.sync.dma_start(gwt[nm][:], wg[:, 768 + i:769 + i])
                for i, nm in enumerate(('bcp', 'bcl', 'bcd')):
                    gwt[nm] = P1.tile([64, 2], f32, name="t_" + nm)
                    nc.sync.dma_start(gwt[nm][:],
                                      wg[0:64, 771 + 2 * i:773 + 2 * i])
                nc.vector.tensor_copy(xh[64:128, :], pb0[:])
                # all-gather the replicated link|device state from shards
                lnb_i = DR.tile([64, (L + N) // 8], f32)
                lnb_g = DR.tile([NCORES, 64, (L + N) // 8], f32)
                nc.gpsimd.dma_start(lnb_i[:], dr['lnts'])
                nc.gpsimd.collective_compute(
                    "AllGather", ALU.bypass,
                    replica_groups=[list(range(NCORES))],
                    ins=[lnb_i.opt()], outs=[lnb_g.opt()])
                nc.sync.dma_start(
                    lnT[:].rearrange("p (c f) -> p c f", c=NCORES),
                    lnb_g[:].rearrange("c p f -> p c f"))
                arb_i = DR.tile([64, L + N], f32)
                arb_o = DR.tile([64, L + N], f32)

                for it in range(ITER):
                    if it > 0:
                        # slot0 = prev path state; restore h into xh
                        nc.vector.tensor_copy(
                            pssT[:, 0:PLOC], pssT[:, T * PLOC:(T + 1) * PLOC])
                        nc.vector.tensor_copy(
                            xh[64:128, :], pssT[:, T * PLOC:(T + 1) * PLOC])
                    for t in range(T):
                        ns = slice((t + 1) * PLOC, (t + 2) * PLOC)
                        isl = slice(t * (PLOC // 8), (t + 1) * (PLOC // 8))
                        nc.gpsimd.ap_gather(
                            gx_, lnT[:], ipk[:, isl],
                            channels=64, num_elems=L + N, d=1, num_idxs=2 * PLOC)
                        nc.vector.tensor_tensor(
                            out=xh[0:64, :], in0=segb[:, 2 * PLOC:3 * PLOC],
                            in1=segb[:, 3 * PLOC:4 * PLOC], op=ALU.add)
                        gru_step(gwt['s2p'], gwt['s3p'], gwt['bzrp'], gwt['bcp'],
                                 xh, pssT[:, t * PLOC:(t + 1) * PLOC],
                                 xh[64:128, :], zg_[:], rg_[:], u_, v_, nb1[:])
                        # archive h_t into the pss table (async DMA copy)
                        nc.sync.dma_start(pssT[:, ns], xh[64:128, :])
                    if it == ITER - 1:
                        break  # final link/device updates are dead code
                    # segment sums over local pss -> partial SL | SN
                    for cc in range(8):
                        qs = slice(cc * 512, (cc + 1) * 512)
                        nc.gpsimd.ap_gather(
                            segb[:], pssT[:], ipk[:, 2048 + cc * 512:
                                                  2048 + (cc + 1) * 512],
                            channels=64, num_elems=NELEM, d=1, num_idxs=4 * PLOC)
                        nc.vector.reduce_sum(
                            slp[:, cc * 512:(cc + 1) * 512],
                            segb[:].rearrange("p (l k) -> p l k", k=K),
                            axis=AX.X)
                    for cc in range(8):
                        qs = slice(cc * 512, (cc + 1) * 512)
                        nc.gpsimd.ap_gather(
                            segb[:], pssT[:], ipk[:, 6144 + cc * 512:
                                                  6144 + (cc + 1) * 512],
                            channels=64, num_elems=NELEM, d=1, num_idxs=4 * PLOC)
                        nc.vector.reduce_sum(
                            slp[:, L + cc * 256:L + (cc + 1) * 256],
                            segb[:].rearrange("p (n k) -> p n k", k=K2),
                            axis=AX.X)
                    nc.gpsimd.dma_start(arb_i[:], slp[:])
                    nc.gpsimd.collective_compute(
                        "AllReduce", ALU.add,
                        replica_groups=[list(range(NCORES))],
                        ins=[arb_i.opt()], outs=[arb_o.opt()])
                    # link GRU (2 chunks of 2048) and device GRU (1 chunk)
                    for ch in range(L // PLOC):
                        s = slice(ch * PLOC, (ch + 1) * PLOC)
                        nc.sync.dma_start(xh[0:64, :], arb_o[:, s])
                        nc.vector.tensor_copy(xh[64:128, :], lnT[:, s])
                        gru_step(gwt['s2l'], gwt['s3l'], gwt['bzrl'], gwt['bcl'],
                                 xh, lnT[:, s], lnT[:, s],
                                 zg_[:], rg_[:], u_, v_, nb1[:])
                    nc.sync.dma_start(xh[0:64, :], arb_o[:, L:L + N])
                    nc.vector.tensor_copy(xh[64:128, :], lnT[:, L:L + N])
                    gru_step(gwt['s2d'], gwt['s3d'], gwt['bzrd'], gwt['bcd'],
                             xh, lnT[:, L:L + N], lnT[:, L:L + N],
                             zg_[:], rg_[:], u_, v_, nb1[:])

            # ---------------- readout ----------------
            with tc.tile_pool(name="ro", bufs=1) as P2:
                h1a = P2.tile([32, PLOC], f32)
                h2a = P2.tile([16, PLOC], f32)
                sp = P2.tile([1, PLOC], f32)
                e1 = P2.tile([1, PLOC], f32)
                ones1 = P2.tile([1, 1], f32)
                wt = P2.tile([1, PLOC], f32)
                da = [P2.tile([1, PLOC], f32, name="da0"),
                      P2.tile([1, PLOC], f32, name="da1")]
                icap_t = P2.tile([1, T * PLOC], f32)
                rwt = {}
                for nm, shp, c0 in (('rw1', [64, 32], 777), ('rw2', [32, 16], 809),
                                    ('rw3', [16, 1], 825), ('rb1', [32, 1], 826),
                                    ('rb2', [16, 1], 827), ('rb3', [1, 1], 828)):
                    rwt[nm] = P2.tile(shp, f32, name="t_" + nm)
                    nc.sync.dma_start(rwt[nm][:], wg[0:shp[0], c0:c0 + shp[1]])
                nc.sync.dma_start(icap_t[:], dr['icap'])
                nc.vector.memset(ones1[:], 1.0)
                for t in range(1, T + 1):
                    ta = PA.tile([128, PLOC], f32, tag="pa")
                    for j in range(NCH):
                        s = slice(j * NB, (j + 1) * NB)
                        ps = slice(t * PLOC + j * NB, t * PLOC + (j + 1) * NB)
                        nc.tensor.matmul(ta[0:32, s], rwt['rw1'][:], pssT[:, ps],
                                         start=True, stop=True)
                    nc.scalar.activation(h1a[:], ta[0:32, :], AF.Relu, bias=rwt['rb1'][:])
                    tb = PB.tile([128, PLOC], f32, tag="pb")
                    for j in range(NCH):
                        s = slice(j * NB, (j + 1) * NB)
                        nc.tensor.matmul(tb[0:16, s], rwt['rw2'][:], h1a[:, s],
                                         start=True, stop=True)
                    nc.scalar.activation(h2a[:], tb[0:16, :], AF.Relu, bias=rwt['rb2'][:])
                    ta2 = PA.tile([128, PLOC], f32, tag="pa")
                    for j in range(NCH):
                        s = slice(j * NB, (j + 1) * NB)
                        nc.tensor.matmul(ta2[0:1, s], rwt['rw3'][:], h2a[:, s],
                                         start=True, stop=True)
                    # softplus(x+b3) = relu(x+b3) + ln(1 + exp(-|x+b3|))
                    nc.scalar.activation(e1[:], ta2[0:1, :], AF.Abs, bias=rwt['rb3'][:])
                    nc.scalar.activation(sp[:], e1[:], AF.Exp, scale=-1.0)
                    nc.scalar.activation(e1[:], sp[:], AF.Ln, bias=ones1[:])
                    nc.scalar.activation(sp[:], ta2[0:1, :], AF.Relu, bias=rwt['rb3'][:])
                    nc.vector.tensor_tensor(out=wt[:], in0=sp[:], in1=e1[:],
                                            op=ALU.add)
                    nc.vector.tensor_tensor(
                        out=e1[:], in0=wt[:],
                        in1=icap_t[:, (t - 1) * PLOC:t * PLOC], op=ALU.mult)
                    if t == 1:
                        nc.vector.tensor_copy(da[1][:], e1[:])
                    else:
                        nc.vector.tensor_tensor(out=da[t % 2][:], in0=da[1 - t % 2][:],
                                                in1=e1[:], op=ALU.add)
                nc.sync.dma_start(out_d, da[T % 2][:])
    nc.compile()
    return nc


def _prog_key():
    if "pkey" not in _NC_CACHE:
        import hashlib, inspect
        _NC_CACHE["pkey"] = hashlib.sha256(
            inspect.getsource(_build_nc).encode()).hexdigest()[:16]
    return _NC_CACHE["pkey"]


def _make_cfg(bir_bytes, arch, in_names, out_names):
    """backend_config for the bass_exec custom call (hook-compatible)."""
    import base64, json, zstandard
    all_in = list(in_names) + list(out_names) + ["partition_id"]
    cfg = {"ant_bir": base64.standard_b64encode(
               zstandard.ZstdCompressor().compress(bir_bytes)).decode(),
           "in_names": all_in, "out_names": list(out_names), "arch": arch}
    return base64.standard_b64encode(json.dumps(cfg).encode()).decode()


def _get_program():
    """(cfg_b64, in_names, out_names, out_specs) — from the /tmp program
    cache when present (skips the 1s Bass trace), else by building."""
    import os, pickle
    path = f"/tmp/bass_gnn_prog_{_prog_key()}.pkl"
    if os.path.exists(path):
        try:
            d = pickle.load(open(path, "rb"))
            return d["cfg"], d["in_names"], d["out_names"], d["out_specs"]
        except Exception:
            pass
    nc = _build_nc()
    from concourse import mybir
    in_names, out_names, out_specs = [], [], []
    for alloc in nc.m.functions[0].allocations:
        if not isinstance(alloc, mybir.MemoryLocationSet):
            continue
        name = alloc.memorylocations[0].name
        if alloc.kind == "ExternalInput":
            if name != "partition_id":
                in_names.append(name)
        elif alloc.kind == "ExternalOutput":
            out_names.append(name)
            out_specs.append((tuple(alloc.tensor_shape),
                              np.dtype(mybir.dt.np(alloc.dtype)).str))
    cfg = _make_cfg(nc.to_json_bytes(), nc.m.arch, in_names, out_names)
    try:
        blob = pickle.dumps(dict(cfg=cfg, in_names=in_names,
                                 out_names=out_names, out_specs=out_specs))
        with open(path + ".tmp", "wb") as fh:
            fh.write(blob)
        os.replace(path + ".tmp", path)
    except Exception:
        pass
    return cfg, in_names, out_names, out_specs


def _local_prims():
    """Local bass_exec/partition-id primitives — the warm path runs with
    zero concourse imports; the backend_config comes pre-baked."""
    import functools
    import jax
    import jax.extend.core
    import jax.interpreters.xla
    from jax._src import effects
    from jax.interpreters import mlir
    from jax._src.interpreters.mlir import custom_call as _cc
    from jax._src.lib.mlir.dialects import mhlo

    pid_p = jax.extend.core.Primitive("partition_id")
    mlir.register_lowering(pid_p, lambda ctx, *_, **__: mhlo.PartitionIdOp().results)
    pid_p.def_impl(functools.partial(jax.interpreters.xla.apply_primitive, pid_p))

    @pid_p.def_abstract_eval
    def _pid_aval(*_, **__):
        return jax.core.ShapedArray((), np.uint32)

    class _KBassEffect(effects.Effect):
        pass

    keff = _KBassEffect()
    mlir.lowerable_effects.add_type(_KBassEffect)
    effects.control_flow_allowed_effects.add_type(_KBassEffect)

    ex_p = jax.extend.core.Primitive("bass_exec")
    ex_p.multiple_results = True

    @ex_p.def_effectful_abstract_eval
    def _ex_aval(*_, out_avals, **__):
        return list(out_avals), {keff}

    def _lowering(ctx, *in_nodes, out_avals, cfg_b64):
        rt = [mlir.aval_to_ir_type(a) for a in ctx.avals_out]
        ol = [list(reversed(range(len(a.shape)))) for a in ctx.avals_in]
        rl = [list(reversed(range(len(a.shape)))) for a in ctx.avals_out]
        fa = {"has_collectives": mlir.ir.StringAttr.get("1")}
        return _cc(
            "bass_exec", operands=in_nodes, result_types=rt,
            operand_layouts=ol, result_layouts=rl, backend_config=cfg_b64,
            extra_attributes={"mhlo.frontend_attributes": mlir.ir.DictAttr.get(fa)},
        ).results

    mlir.register_lowering(ex_p, _lowering, platform="neuron")
    return pid_p, ex_p


def _ensure_hook(cdir, force=False):
    """Install the bass_exec NEFF-compile hook. Skipped on the warm path
    when a prior successful run left a marker (executable comes from the
    jax persistent cache, so no compile happens)."""
    import os
    if _NC_CACHE.get("hook"):
        return
    if not force and os.path.exists(
            os.path.join(cdir, f"ok_{_prog_key()}")):
        return
    from concourse import bass2jax
    bass2jax.install_neuronx_cc_hook()
    _NC_CACHE["hook"] = True


def _prepare(cdir):
    """Everything input-independent: jax config, program load, hook
    decision, jit construction, AOT compile (persistent-cache load).
    Runs in a background thread started at module import."""
    import os
    import jax
    try:
        os.makedirs(cdir, exist_ok=True)
        jax.config.update("jax_compilation_cache_dir", cdir)
        jax.config.update("jax_persistent_cache_min_compile_time_secs", 0.0)
        jax.config.update("jax_persistent_cache_min_entry_size_bytes", 0)
    except Exception:
        pass
    prog_cached = os.path.exists(f"/tmp/bass_gnn_prog_{_prog_key()}.pkl")
    prog = _get_program()
    _NC_CACHE["prog"] = prog
    _ensure_hook(cdir, force=not prog_cached)
    _NC_CACHE["compiled"] = _compile_exec(cdir, *prog)


def _compile_exec(cdir, cfg_b64, in_names, out_names, out_specs):
    import jax
    import numpy as np
    from jax.experimental.shard_map import shard_map
    from jax.sharding import Mesh, PartitionSpec

    pid_p, ex_p = _local_prims()
    out_avals = [jax.core.ShapedArray(s, np.dtype(d)) for s, d in out_specs]
    n_params = len(in_names)
    n_outs = len(out_names)

    def _body(*args):
        operands = list(args)
        operands.append(pid_p.bind().reshape(1, 1))
        return tuple(ex_p.bind(
            *operands, out_avals=tuple(out_avals), cfg_b64=cfg_b64))

    devices = jax.devices()[:NCORES]
    mesh = Mesh(np.asarray(devices), ("core",))
    sharded = jax.jit(
        shard_map(_body, mesh=mesh,
                  in_specs=(PartitionSpec("core"),) * (n_params + n_outs),
                  out_specs=(PartitionSpec("core"),) * n_outs,
                  check_rep=False),
        donate_argnums=tuple(range(n_params, n_params + n_outs)),
        keep_unused=True,
    )
    global_avals = _NC_CACHE["gavals"]
    placeholders = [jax.ShapeDtypeStruct((NCORES * s[0], *s[1:]), d)
                    for s, d in global_avals]
    compiled = sharded.lower(*placeholders).compile()
    return compiled


def _run_spmd(in_maps, cdir):
    import os
    import jax
    import numpy as np

    cfg_b64, in_names, out_names, out_specs = _NC_CACHE["prog"]
    out_avals = [(tuple(s), np.dtype(d)) for s, d in out_specs]
    concat_in = [
        np.concatenate([in_maps[c][name] for c in range(NCORES)], axis=0)
        for name in in_names
    ]

    def _call():
        concat_zeros = [
            np.zeros((NCORES * s[0], *s[1:]), d) for s, d in out_avals
        ]
        out_arrs = _NC_CACHE["compiled"](*concat_in, *concat_zeros)
        for a in out_arrs:
            for sh in a.addressable_shards:
                sh.data.copy_to_host_async()
        return {
            name: np.asarray(out_arrs[i]).reshape(NCORES, *out_avals[i][0])
            for i, name in enumerate(out_names)
        }

    try:
        res = _call()
    except Exception:
        # compile needed but hook absent (stale marker) -> install, redo
        _ensure_hook(cdir, force=True)
        _NC_CACHE["compiled"] = _compile_exec(cdir, *_NC_CACHE["prog"])
        res = _call()
    try:
        open(os.path.join(cdir, f"ok_{_prog_key()}"), "w").close()
    except Exception:
        pass
    return res


def kernel(**inputs):
    import os
    cdir = os.environ.get("JAX_COMPILATION_CACHE_DIR", "/tmp/jaxcache")
    in_maps = _host_prep(inputs)
    if _BG_THREAD is not None:
        _BG_THREAD.join()
    if "compiled" not in _NC_CACHE:
        _prepare(cdir)   # background warmup failed or absent: do it now
    outs = _run_spmd(in_maps, cdir)
    full = np.ascontiguousarray(
        outs["out"].reshape(NCORES, PLOC).reshape(P)[:, None])
    kernel._last_res = None
    return full


# The per-core global input shapes, in _get_program's in_names order,
# then the donated output buffers (kept in sync with _build_nc's tensors).
import ml_dtypes as _mld
_NC_CACHE["gavals"] = [
    ((64, PLOC), np.dtype(_mld.bfloat16)), ((64, (L + N) // 8), np.dtype('<f4')),
    ((16, 832), np.dtype('<f4')),
    ((16, (2 * T * PLOC + L * K + N * K2) // 16), np.dtype('<i2')),
    ((1, T * PLOC), np.dtype('<f4')), ((1, PLOC), np.dtype('<f4')),
]


def _bg_warmup():
    import os
    try:
        _prepare(os.environ.get("JAX_COMPILATION_CACHE_DIR", "/tmp/jaxcache"))
    except Exception:
        _NC_CACHE.pop("compiled", None)


_BG_THREAD = None
try:
    import threading
    _BG_THREAD = threading.Thread(target=_bg_warmup, daemon=True)
    _BG_THREAD.start()
except Exception:
    _BG_THREAD = None



# revision 7
# speedup vs baseline: 1.2812x; 1.2812x over previous
"""Trainium2 Bass kernel for nn_Baseline_mb_24189255811183 (gnn_message_passing).

Full on-device SPMD implementation on 8 NeuronCores.

Sharding: paths are sharded 8-ways (2048 paths/core) per the sharding hint;
link_state [L,64] and device_state [N,64] are replicated on every core; the
per-link/per-node path-state segment reductions are computed as local partial
sums and AllReduce'd each message-passing iteration; parameters replicated.

Everything lives transposed in SBUF (features on partitions):
  - pssT [64, 9*2048]  path-state sequence table (slot = t*2048 + p).
  - x gathers (link_state[link_to_path], device_state[node_to_path]) and the
    segment-sum gathers (pss[pl0,pl1], pss[pn0,pn1]) run on the GPSIMD engine
    via ap_gather (SBUF free-dim gather; indices int16, wrapped 16-partition
    layout, static across iterations, precomputed on host).
  - Each GRU step stacks its two matmul inputs in one [128, 2048] tile
    (parts 0:64 = x, 64:128 = h) so one stationary [128,128] matrix computes
    x@wx_zr + h@wh_zr (gates) and a block-diagonal one computes (xc | hc).
    Gate biases ride the ACT ops' per-partition bias vectors.
  - The x-gather uses one merged [64, L+N] link|device table per step;
    matmuls chunk the free dim at 512 (one PSUM bank of f32).
  - The last iteration's link/device GRU updates are dead code and skipped.
  - Readout (relu MLP -> softplus -> capacity-weighted hop sum) fused on
    device; per-core output is the [2048] delay shard.

Host does only the cheap O(P*D) encoders and index packing (~0.05 s).

Launch-path optimizations (the axon relay moves inputs at ~100 MB/s and
charges one round trip per serial fetch, so bytes and tensor count matter):
  - inputs packed into 5 tensors/core (~0.9 MB): bf16 initial path state,
    1/8 shards of the link|device table and of the weight pack (both
    AllGathered on device; the weight pack shards by rows so the gather's
    flat concat reproduces the [128, 832] layout exactly), one int16 index
    pack (16-row wrap, replicated to 64 on device).
  - a local bass_exec/partition-id primitive pair with a pre-baked
    backend_config, so the warm path never imports concourse (saves the
    0.3 s import and the per-process zstd/b64 of the BIR); the jitted
    executable is cached in-process and the 8 output shards are fetched
    with overlapped async copies.
  - /tmp program cache (backend_config keyed on the builder source hash)
    skips the Bass re-trace, and the JAX persistent compilation cache skips
    the NEFF compile, in warm fresh processes. The compile hook (and
    concourse) is imported only when a marker shows a compile is needed;
    a failed compile falls back to installing the hook and retrying.
    Every cache layer degrades gracefully to a full build.
"""
import sys
sys.path.insert(0, '/opt/trn_rl_repo')
import numpy as np
import os as _os, time as _time
_T0 = _time.time()
_DBG = bool(_os.environ.get("KBENCH_DEBUG"))


def _dbg(msg):
    if _DBG:
        print(f"[kdbg +{_time.time() - _T0:7.3f}s] {msg}", file=sys.stderr, flush=True)

P, T, L, K, N, K2, M, D = 16384, 8, 4096, 16, 2048, 32, 8, 64
ITER = 8
NCORES = 8
PLOC = P // NCORES              # 2048 paths per core
NSLOT = (T + 1) * PLOC          # 18432 pss slots (slot = t*PLOC + p)
ZSLOT = NSLOT                   # zero row for non-local segment entries
NELEM = NSLOT + 16              # padded ap_gather table size

_NC_CACHE = {}


# ---------------------------------------------------------------- host math
def _relu(v):
    return np.maximum(v, 0.0)


def _mlp2(x, w1, b1, w2, b2):
    return _relu(_relu(x @ w1 + b1) @ w2 + b2)


def _wrap_idx(idx_list):
    """int16 index list -> [64, n/16] wrapped layout for ap_gather."""
    n = idx_list.shape[0]
    w = idx_list.reshape(n // 16, 16).T.astype(np.int16)   # [16, n/16]
    return np.tile(w, (4, 1))                              # [64, n/16]


def _gru_mats(wx, bx, wh, bh):
    """Stationary matrices + bias vectors for the stacked-input GRU."""
    wx = np.asarray(wx, np.float32); wh = np.asarray(wh, np.float32)
    bx = np.asarray(bx, np.float32); bh = np.asarray(bh, np.float32)
    s2 = np.ascontiguousarray(np.vstack([wx[:, 0:128], wh[:, 0:128]]))  # [128,128]
    s3 = np.zeros((128, 128), np.float32)
    s3[0:64, 0:64] = wx[:, 128:192]
    s3[64:128, 64:128] = wh[:, 128:192]
    bzr = (bx + bh)[0:128].reshape(128, 1).astype(np.float32)
    bc = np.stack([bh[128:192], bx[128:192]], axis=1).astype(np.float32)  # [64,2]
    return s2, s3, bzr, bc


def _host_prep(inp):
    f = lambda k: np.ascontiguousarray(np.asarray(inp[k], np.float32))
    ft, fp, fps, cap = f('flow_traffic'), f('flow_packets'), f('flow_packet_size'), f('link_capacity')
    ltp, ntp = np.asarray(inp['link_to_path']), np.asarray(inp['node_to_path'])
    ptl, ptn, ltn = np.asarray(inp['path_to_link']), np.asarray(inp['path_to_node']), np.asarray(inp['link_to_node'])

    ldt = (np.asarray(inp['link_device_type']) == 0).astype(np.float32)[:, None]
    load = ft[ptl[:, :, 0], 0].sum(1)[:, None] / (cap * 1e9)
    path_state = _mlp2(np.concatenate([ft * 1e-4, fp * 1e-3, fps * 1e-3], 1),
                       f('pe_w1'), f('pe_b1'), f('pe_w2'), f('pe_b2'))
    link_state = _mlp2(np.concatenate([cap * 1e-2, load, ldt], 1),
                       f('le_w1'), f('le_b1'), f('le_w2'), f('le_b2'))
    dlm = link_state[ltn].sum(1).mean(1, keepdims=True)
    dev_enc = (np.asarray(inp['nodes']) == 0).astype(np.float32)[:, None]
    device_state = _mlp2(np.concatenate([dev_enc, dlm], 1),
                         f('de_w1'), f('de_b1'), f('de_w2'), f('de_b2'))

    s2p, s3p, bzrp, bcp = _gru_mats(inp['pgru_wx'], inp['pgru_bx'], inp['pgru_wh'], inp['pgru_bh'])
    s2l, s3l, bzrl, bcl = _gru_mats(inp['lgru_wx'], inp['lgru_bx'], inp['lgru_wh'], inp['lgru_bh'])
    s2d, s3d, bzrd, bcd = _gru_mats(inp['dgru_wx'], inp['dgru_bx'], inp['dgru_wh'], inp['dgru_bh'])
    rw1, rw2, rw3 = f('ro_w1'), f('ro_w2'), f('ro_w3')
    rb1 = f('ro_b1').reshape(32, 1); rb2 = f('ro_b2').reshape(16, 1)
    rb3 = f('ro_b3').reshape(1, 1)

    lnT = np.ascontiguousarray(
        np.concatenate([link_state.T, device_state.T], axis=1))  # [64, 6144]
    # one [128, 832] pack of every replicated parameter tensor
    wpack = np.zeros((128, 832), np.float32)
    for i, s in enumerate((s2p, s3p, s2l, s3l, s2d, s3d)):
        wpack[:, i * 128:(i + 1) * 128] = s
    wpack[:, 768:769] = bzrp; wpack[:, 769:770] = bzrl; wpack[:, 770:771] = bzrd
    wpack[0:64, 771:773] = bcp; wpack[0:64, 773:775] = bcl; wpack[0:64, 775:777] = bcd
    wpack[0:64, 777:809] = rw1; wpack[0:32, 809:825] = rw2; wpack[0:16, 825:826] = rw3
    wpack[0:32, 826:827] = rb1; wpack[0:16, 827:828] = rb2; wpack[0:1, 828:829] = rb3

    import ml_dtypes
    pl0, pl1 = ptl[:, :, 0].astype(np.int32), ptl[:, :, 1].astype(np.int32)
    pn0, pn1 = ptn[:, :, 0].astype(np.int32), ptn[:, :, 1].astype(np.int32)
    glb = pl1 * PLOC + pl0          # local idx before rebase, [L, K]
    gnb = pn1 * PLOC + pn0          # [N, K2]
    icap_all = (1.0 / cap[ltp, 0]).astype(np.float32)      # [P, T]

    in_maps = []
    for c in range(NCORES):
        lo = c * PLOC
        sl = slice(lo, lo + PLOC)
        ps0 = path_state[sl].T.astype(ml_dtypes.bfloat16)
        # merged x gather indices: per step t, [link idxs | node idxs + L]
        ixc = np.concatenate([ltp[sl].T.astype(np.int32),
                              ntp[sl].T + np.int32(L)], axis=1)  # [T, 2*PLOC]
        # segment-sum indices over the local pss table
        gl = np.where((pl0 >= lo) & (pl0 < lo + PLOC), glb - lo, ZSLOT)
        gn = np.where((pn0 >= lo) & (pn0 < lo + PLOC), gnb - lo, ZSLOT)
        # one [16, 10240] idx pack, single 16-row wrap (device replicates x4)
        ip = np.concatenate([ixc.reshape(-1), gl.reshape(-1), gn.reshape(-1)])
        ipack = ip.reshape(-1, 16).T.astype(np.int16)      # [16, 10240]
        icap = np.ascontiguousarray(icap_all[sl].T).reshape(1, T * PLOC)
        lnts = lnT[:, c * 768:(c + 1) * 768]
        wps = wpack[16 * c:16 * (c + 1), :]
        in_maps.append(dict(ps0=ps0, lnts=lnts, wps=wps,
                            ipack=ipack, icap=icap))
    return in_maps


# ------------------------------------------------------------- device kernel
def _build_nc():
    import concourse.bacc as bacc
    import concourse.tile as tile
    import concourse.mybir as mybir

    f32, bf16, i16 = mybir.dt.float32, mybir.dt.bfloat16, mybir.dt.int16
    AF = mybir.ActivationFunctionType
    ALU = mybir.AluOpType
    AX = mybir.AxisListType

    nc = bacc.Bacc("TRN2", target_bir_lowering=False, debug=False,
                   num_devices=NCORES)
    dr = {}
    for name, shape, dt in (
        ('ps0', [64, PLOC], bf16), ('lnts', [64, (L + N) // 8], f32),
        ('wps', [16, 832], f32),
        ('ipack', [16, (2 * T * PLOC + L * K + N * K2) // 16], i16),
        ('icap', [1, T * PLOC], f32),
    ):
        dr[name] = nc.dram_tensor(name, shape, dt, kind="ExternalInput").ap()
    out_d = nc.dram_tensor("out", [1, PLOC], f32, kind="ExternalOutput").ap()

    NB = 512     # matmul free-dim chunk (one PSUM bank of f32)
    NCH = PLOC // NB

    with tile.TileContext(nc) as tc:
        with (
            tc.tile_pool(name="persist", bufs=1) as P0,
            tc.tile_pool(name="psumA", bufs=1, space="PSUM") as PA,
            tc.tile_pool(name="psumB", bufs=1, space="PSUM") as PB,
            tc.tile_pool(name="dram", bufs=1, space="DRAM") as DR,
        ):
            pssT = P0.tile([64, NELEM], f32)
            pb0 = P0.tile([64, PLOC], bf16)
            nc.sync.dma_start(pb0[:], dr['ps0'])
            nc.vector.tensor_copy(pssT[:, 0:PLOC], pb0[:])
            nc.vector.memset(pssT[:, NSLOT:NELEM], 0.0)

            def gru_step(s2, s3, bzr, bc, xh, hprev0, hout, zg, rg, u, v, scr):
                """GRU update, stacked layout: xh [128, W] = (x | h).

                hprev0 [64, W] base-partition-0 copy of h; hout [64, W];
                zg, rg [64, W] bf16; u, v, scr [64, W] f32.
                """
                ta = PA.tile([128, PLOC], f32, tag="pa")   # (z~ | r~)
                tb = PB.tile([128, PLOC], f32, tag="pb")   # (xc | hc)
                for j in range(NCH):
                    s = slice(j * NB, (j + 1) * NB)
                    nc.tensor.matmul(ta[:, s], s2[:], xh[:, s], start=True, stop=True)
                    nc.tensor.matmul(tb[:, s], s3[:], xh[:, s], start=True, stop=True)
                nc.scalar.activation(zg, ta[0:64, :], AF.Sigmoid, bias=bzr[0:64, :])
                nc.scalar.activation(rg, ta[64:128, :], AF.Sigmoid, bias=bzr[64:128, :])
                nc.scalar.activation(u, tb[64:128, :], AF.Identity, bias=bc[:, 0:1])
                nc.vector.tensor_tensor(out=v, in0=rg, in1=u, op=ALU.mult)
                nc.vector.tensor_tensor(out=u, in0=tb[0:64, :], in1=v, op=ALU.add)
                nc.scalar.activation(v, u, AF.Tanh, bias=bc[:, 1:2])   # v = c
                nc.vector.tensor_tensor(out=u, in0=hprev0, in1=v, op=ALU.subtract)
                nc.vector.tensor_tensor(out=scr, in0=zg, in1=u, op=ALU.mult)
                nc.vector.tensor_tensor(out=hout, in0=v, in1=scr, op=ALU.add)

            with tc.tile_pool(name="iter", bufs=1) as P1:
                lnT = P1.tile([64, L + N], f32)     # link | device states
                slp = P1.tile([64, L + N], f32)     # partial SL | SN
                xh = P1.tile([128, PLOC], f32)      # (x | h) stacked
                nb1 = P1.tile([64, PLOC], f32)
                segb = P1.tile([64, 4 * PLOC], f32)
                u_ = segb[:, 0:PLOC]
                v_ = segb[:, PLOC:2 * PLOC]
                gx_ = segb[:, 2 * PLOC:4 * PLOC]    # merged x-gather landing
                zg_ = P1.tile([64, PLOC], bf16)
                rg_ = P1.tile([64, PLOC], bf16)
                NIP = (2 * T * PLOC + L * K + N * K2) // 16   # 10240
                ipk = P1.tile([64, NIP], i16)
                for q in range(4):
                    nc.sync.dma_start(ipk[16 * q:16 * (q + 1), :], dr['ipack'])
                # all-gather the weight pack from 16-row shards (row concat)
                wpb_i = DR.tile([16, 832], f32)
                wg = DR.tile([128, 832], f32)
                nc.gpsimd.dma_start(wpb_i[:], dr['wps'])
                nc.gpsimd.collective_compute(
                    "AllGather", ALU.bypass,
                    replica_groups=[list(range(NCORES))],
                    ins=[wpb_i.opt()], outs=[wg.opt()])
                WOF = {'s2p': 0, 's3p': 128, 's2l': 256, 's3l': 384,
                       's2d': 512, 's3d': 640}
                gwt = {}
                for nm, off in WOF.items():
                    gwt[nm] = P1.tile([128, 128], f32, name="t_" + nm)
                    nc.sync.dma_start(gwt[nm][:], wg[:, off:off + 128])
                for i, nm in enumerate(('bzrp', 'bzrl', 'bzrd')):
                    gwt[nm] = P1.tile([128, 1], f32, name="t_" + nm)
                    nc.sync.dma_start(gwt[nm][:], wg[:, 768 + i:769 + i])
                for i, nm in enumerate(('bcp', 'bcl', 'bcd')):
                    gwt[nm] = P1.tile([64, 2], f32, name="t_" + nm)
                    nc.sync.dma_start(gwt[nm][:],
                                      wg[0:64, 771 + 2 * i:773 + 2 * i])
                nc.vector.tensor_copy(xh[64:128, :], pb0[:])
                # all-gather the replicated link|device state from shards
                lnb_i = DR.tile([64, (L + N) // 8], f32)
                lnb_g = DR.tile([NCORES, 64, (L + N) // 8], f32)
                nc.gpsimd.dma_start(lnb_i[:], dr['lnts'])
                nc.gpsimd.collective_compute(
                    "AllGather", ALU.bypass,
                    replica_groups=[list(range(NCORES))],
                    ins=[lnb_i.opt()], outs=[lnb_g.opt()])
                nc.sync.dma_start(
                    lnT[:].rearrange("p (c f) -> p c f", c=NCORES),
                    lnb_g[:].rearrange("c p f -> p c f"))
                arb_i = DR.tile([64, L + N], f32)
                arb_o = DR.tile([64, L + N], f32)

                for it in range(ITER):
                    if it > 0:
                        # slot0 = prev path state; restore h into xh
                        nc.vector.tensor_copy(
                            pssT[:, 0:PLOC], pssT[:, T * PLOC:(T + 1) * PLOC])
                        nc.vector.tensor_copy(
                            xh[64:128, :], pssT[:, T * PLOC:(T + 1) * PLOC])
                    for t in range(T):
                        ns = slice((t + 1) * PLOC, (t + 2) * PLOC)
                        isl = slice(t * (PLOC // 8), (t + 1) * (PLOC // 8))
                        nc.gpsimd.ap_gather(
                            gx_, lnT[:], ipk[:, isl],
                            channels=64, num_elems=L + N, d=1, num_idxs=2 * PLOC)
                        nc.vector.tensor_tensor(
                            out=xh[0:64, :], in0=segb[:, 2 * PLOC:3 * PLOC],
                            in1=segb[:, 3 * PLOC:4 * PLOC], op=ALU.add)
                        gru_step(gwt['s2p'], gwt['s3p'], gwt['bzrp'], gwt['bcp'],
                                 xh, pssT[:, t * PLOC:(t + 1) * PLOC],
                                 xh[64:128, :], zg_[:], rg_[:], u_, v_, nb1[:])
                        # archive h_t into the pss table (async DMA copy)
                        nc.sync.dma_start(pssT[:, ns], xh[64:128, :])
                    if it == ITER - 1:
                        break  # final link/device updates are dead code
                    # segment sums over local pss -> partial SL | SN
                    for cc in range(8):
                        qs = slice(cc * 512, (cc + 1) * 512)
                        nc.gpsimd.ap_gather(
                            segb[:], pssT[:], ipk[:, 2048 + cc * 512:
                                                  2048 + (cc + 1) * 512],
                            channels=64, num_elems=NELEM, d=1, num_idxs=4 * PLOC)
                        nc.vector.reduce_sum(
                            slp[:, cc * 512:(cc + 1) * 512],
                            segb[:].rearrange("p (l k) -> p l k", k=K),
                            axis=AX.X)
                    for cc in range(8):
                        qs = slice(cc * 512, (cc + 1) * 512)
                        nc.gpsimd.ap_gather(
                            segb[:], pssT[:], ipk[:, 6144 + cc * 512:
                                                  6144 + (cc + 1) * 512],
                            channels=64, num_elems=NELEM, d=1, num_idxs=4 * PLOC)
                        nc.vector.reduce_sum(
                            slp[:, L + cc * 256:L + (cc + 1) * 256],
                            segb[:].rearrange("p (n k) -> p n k", k=K2),
                            axis=AX.X)
                    nc.gpsimd.dma_start(arb_i[:], slp[:])
                    nc.gpsimd.collective_compute(
                        "AllReduce", ALU.add,
                        replica_groups=[list(range(NCORES))],
                        ins=[arb_i.opt()], outs=[arb_o.opt()])
                    # link GRU (2 chunks of 2048) and device GRU (1 chunk)
                    for ch in range(L // PLOC):
                        s = slice(ch * PLOC, (ch + 1) * PLOC)
                        nc.sync.dma_start(xh[0:64, :], arb_o[:, s])
                        nc.vector.tensor_copy(xh[64:128, :], lnT[:, s])
                        gru_step(gwt['s2l'], gwt['s3l'], gwt['bzrl'], gwt['bcl'],
                                 xh, lnT[:, s], lnT[:, s],
                                 zg_[:], rg_[:], u_, v_, nb1[:])
                    nc.sync.dma_start(xh[0:64, :], arb_o[:, L:L + N])
                    nc.vector.tensor_copy(xh[64:128, :], lnT[:, L:L + N])
                    gru_step(gwt['s2d'], gwt['s3d'], gwt['bzrd'], gwt['bcd'],
                             xh, lnT[:, L:L + N], lnT[:, L:L + N],
                             zg_[:], rg_[:], u_, v_, nb1[:])

            # ---------------- readout ----------------
            with tc.tile_pool(name="ro", bufs=1) as P2:
                h1a = P2.tile([32, PLOC], f32)
                h2a = P2.tile([16, PLOC], f32)
                sp = P2.tile([1, PLOC], f32)
                e1 = P2.tile([1, PLOC], f32)
                ones1 = P2.tile([1, 1], f32)
                wt = P2.tile([1, PLOC], f32)
                da = [P2.tile([1, PLOC], f32, name="da0"),
                      P2.tile([1, PLOC], f32, name="da1")]
                icap_t = P2.tile([1, T * PLOC], f32)
                rwt = {}
                for nm, shp, c0 in (('rw1', [64, 32], 777), ('rw2', [32, 16], 809),
                                    ('rw3', [16, 1], 825), ('rb1', [32, 1], 826),
                                    ('rb2', [16, 1], 827), ('rb3', [1, 1], 828)):
                    rwt[nm] = P2.tile(shp, f32, name="t_" + nm)
                    nc.sync.dma_start(rwt[nm][:], wg[0:shp[0], c0:c0 + shp[1]])
                nc.sync.dma_start(icap_t[:], dr['icap'])
                nc.vector.memset(ones1[:], 1.0)
                for t in range(1, T + 1):
                    ta = PA.tile([128, PLOC], f32, tag="pa")
                    for j in range(NCH):
                        s = slice(j * NB, (j + 1) * NB)
                        ps = slice(t * PLOC + j * NB, t * PLOC + (j + 1) * NB)
                        nc.tensor.matmul(ta[0:32, s], rwt['rw1'][:], pssT[:, ps],
                                         start=True, stop=True)
                    nc.scalar.activation(h1a[:], ta[0:32, :], AF.Relu, bias=rwt['rb1'][:])
                    tb = PB.tile([128, PLOC], f32, tag="pb")
                    for j in range(NCH):
                        s = slice(j * NB, (j + 1) * NB)
                        nc.tensor.matmul(tb[0:16, s], rwt['rw2'][:], h1a[:, s],
                                         start=True, stop=True)
                    nc.scalar.activation(h2a[:], tb[0:16, :], AF.Relu, bias=rwt['rb2'][:])
                    ta2 = PA.tile([128, PLOC], f32, tag="pa")
                    for j in range(NCH):
                        s = slice(j * NB, (j + 1) * NB)
                        nc.tensor.matmul(ta2[0:1, s], rwt['rw3'][:], h2a[:, s],
                                         start=True, stop=True)
                    # softplus(x+b3) = relu(x+b3) + ln(1 + exp(-|x+b3|))
                    nc.scalar.activation(e1[:], ta2[0:1, :], AF.Abs, bias=rwt['rb3'][:])
                    nc.scalar.activation(sp[:], e1[:], AF.Exp, scale=-1.0)
                    nc.scalar.activation(e1[:], sp[:], AF.Ln, bias=ones1[:])
                    nc.scalar.activation(sp[:], ta2[0:1, :], AF.Relu, bias=rwt['rb3'][:])
                    nc.vector.tensor_tensor(out=wt[:], in0=sp[:], in1=e1[:],
                                            op=ALU.add)
                    nc.vector.tensor_tensor(
                        out=e1[:], in0=wt[:],
                        in1=icap_t[:, (t - 1) * PLOC:t * PLOC], op=ALU.mult)
                    if t == 1:
                        nc.vector.tensor_copy(da[1][:], e1[:])
                    else:
                        nc.vector.tensor_tensor(out=da[t % 2][:], in0=da[1 - t % 2][:],
                                                in1=e1[:], op=ALU.add)
                nc.sync.dma_start(out_d, da[T % 2][:])
    nc.compile()
    return nc


def _prog_key():
    if "pkey" not in _NC_CACHE:
        import hashlib, inspect
        _NC_CACHE["pkey"] = hashlib.sha256(
            inspect.getsource(_build_nc).encode()).hexdigest()[:16]
    return _NC_CACHE["pkey"]


def _make_cfg(bir_bytes, arch, in_names, out_names):
    """backend_config for the bass_exec custom call (hook-compatible)."""
    import base64, json, zstandard
    all_in = list(in_names) + list(out_names) + ["partition_id"]
    cfg = {"ant_bir": base64.standard_b64encode(
               zstandard.ZstdCompressor().compress(bir_bytes)).decode(),
           "in_names": all_in, "out_names": list(out_names), "arch": arch}
    return base64.standard_b64encode(json.dumps(cfg).encode()).decode()


def _get_program():
    """(cfg_b64, in_names, out_names, out_specs) — from the /tmp program
    cache when present (skips the 1s Bass trace), else by building."""
    import os, pickle
    path = f"/tmp/bass_gnn_prog_{_prog_key()}.pkl"
    if os.path.exists(path):
        try:
            d = pickle.load(open(path, "rb"))
            return d["cfg"], d["in_names"], d["out_names"], d["out_specs"]
        except Exception:
            pass
    nc = _build_nc()
    from concourse import mybir
    in_names, out_names, out_specs = [], [], []
    for alloc in nc.m.functions[0].allocations:
        if not isinstance(alloc, mybir.MemoryLocationSet):
            continue
        name = alloc.memorylocations[0].name
        if alloc.kind == "ExternalInput":
            if name != "partition_id":
                in_names.append(name)
        elif alloc.kind == "ExternalOutput":
            out_names.append(name)
            out_specs.append((tuple(alloc.tensor_shape),
                              np.dtype(mybir.dt.np(alloc.dtype)).str))
    cfg = _make_cfg(nc.to_json_bytes(), nc.m.arch, in_names, out_names)
    try:
        blob = pickle.dumps(dict(cfg=cfg, in_names=in_names,
                                 out_names=out_names, out_specs=out_specs))
        with open(path + ".tmp", "wb") as fh:
            fh.write(blob)
        os.replace(path + ".tmp", path)
    except Exception:
        pass
    return cfg, in_names, out_names, out_specs


def _local_prims():
    """Local bass_exec/partition-id primitives — the warm path runs with
    zero concourse imports; the backend_config comes pre-baked."""
    import functools
    import jax
    import jax.extend.core
    import jax.interpreters.xla
    from jax._src import effects
    from jax.interpreters import mlir
    from jax._src.interpreters.mlir import custom_call as _cc
    from jax._src.lib.mlir.dialects import mhlo

    pid_p = jax.extend.core.Primitive("partition_id")
    mlir.register_lowering(pid_p, lambda ctx, *_, **__: mhlo.PartitionIdOp().results)
    pid_p.def_impl(functools.partial(jax.interpreters.xla.apply_primitive, pid_p))

    @pid_p.def_abstract_eval
    def _pid_aval(*_, **__):
        return jax.core.ShapedArray((), np.uint32)

    class _KBassEffect(effects.Effect):
        pass

    keff = _KBassEffect()
    mlir.lowerable_effects.add_type(_KBassEffect)
    effects.control_flow_allowed_effects.add_type(_KBassEffect)

    ex_p = jax.extend.core.Primitive("bass_exec")
    ex_p.multiple_results = True

    @ex_p.def_effectful_abstract_eval
    def _ex_aval(*_, out_avals, **__):
        return list(out_avals), {keff}

    def _lowering(ctx, *in_nodes, out_avals, cfg_b64):
        rt = [mlir.aval_to_ir_type(a) for a in ctx.avals_out]
        ol = [list(reversed(range(len(a.shape)))) for a in ctx.avals_in]
        rl = [list(reversed(range(len(a.shape)))) for a in ctx.avals_out]
        fa = {"has_collectives": mlir.ir.StringAttr.get("1")}
        return _cc(
            "bass_exec", operands=in_nodes, result_types=rt,
            operand_layouts=ol, result_layouts=rl, backend_config=cfg_b64,
            extra_attributes={"mhlo.frontend_attributes": mlir.ir.DictAttr.get(fa)},
        ).results

    mlir.register_lowering(ex_p, _lowering, platform="neuron")
    return pid_p, ex_p


def _ensure_hook(cdir, force=False):
    """Install the bass_exec NEFF-compile hook. Skipped on the warm path
    when a prior successful run left a marker (executable comes from the
    jax persistent cache, so no compile happens)."""
    import os
    if _NC_CACHE.get("hook"):
        return
    if not force and os.path.exists(
            os.path.join(cdir, f"ok_{_prog_key()}")):
        return
    from concourse import bass2jax
    bass2jax.install_neuronx_cc_hook()
    _NC_CACHE["hook"] = True


def _prepare(cdir):
    """Everything input-independent: jax config, program load, hook
    decision, jit construction, AOT compile (persistent-cache load).
    Runs in a background thread started at module import."""
    import os
    _dbg("prepare: start (pre jax import)")
    import jax
    _dbg("prepare: jax imported")
    try:
        os.makedirs(cdir, exist_ok=True)
        jax.config.update("jax_compilation_cache_dir", cdir)
        jax.config.update("jax_persistent_cache_min_compile_time_secs", 0.0)
        jax.config.update("jax_persistent_cache_min_entry_size_bytes", 0)
    except Exception:
        pass
    prog_cached = os.path.exists(f"/tmp/bass_gnn_prog_{_prog_key()}.pkl")
    prog = _get_program()
    _dbg("prepare: program loaded")
    _NC_CACHE["prog"] = prog
    _ensure_hook(cdir, force=not prog_cached)
    _dbg("prepare: hook done")
    _NC_CACHE["compiled"] = _compile_exec(cdir, *prog)
    _dbg("prepare: compiled ready")


def _compile_exec(cdir, cfg_b64, in_names, out_names, out_specs):
    import jax
    import numpy as np
    from jax.experimental.shard_map import shard_map
    from jax.sharding import Mesh, PartitionSpec

    _dbg(f"compile_exec: start; devices visible: {len(jax.devices())}")
    pid_p, ex_p = _local_prims()
    out_avals = [jax.core.ShapedArray(s, np.dtype(d)) for s, d in out_specs]
    n_params = len(in_names)
    n_outs = len(out_names)

    def _body(*args):
        operands = list(args)
        operands.append(pid_p.bind().reshape(1, 1))
        return tuple(ex_p.bind(
            *operands, out_avals=tuple(out_avals), cfg_b64=cfg_b64))

    devices = jax.devices()[:NCORES]
    mesh = Mesh(np.asarray(devices), ("core",))
    sharded = jax.jit(
        shard_map(_body, mesh=mesh,
                  in_specs=(PartitionSpec("core"),) * (n_params + n_outs),
                  out_specs=(PartitionSpec("core"),) * n_outs,
                  check_rep=False),
        donate_argnums=tuple(range(n_params, n_params + n_outs)),
        keep_unused=True,
    )
    global_avals = _NC_CACHE["gavals"]
    placeholders = [jax.ShapeDtypeStruct((NCORES * s[0], *s[1:]), d)
                    for s, d in global_avals]
    _dbg("compile_exec: lowering")
    lowered = sharded.lower(*placeholders)
    _dbg("compile_exec: lowered; compiling")
    compiled = lowered.compile()
    _dbg("compile_exec: compiled")
    return compiled


def _run_spmd(in_maps, cdir):
    import os
    import jax
    import numpy as np

    cfg_b64, in_names, out_names, out_specs = _NC_CACHE["prog"]
    out_avals = [(tuple(s), np.dtype(d)) for s, d in out_specs]
    concat_in = [
        np.concatenate([in_maps[c][name] for c in range(NCORES)], axis=0)
        for name in in_names
    ]
    _dbg("run_spmd: inputs concatenated")

    def _call():
        concat_zeros = [
            np.zeros((NCORES * s[0], *s[1:]), d) for s, d in out_avals
        ]
        _dbg("run_spmd: dispatching")
        out_arrs = _NC_CACHE["compiled"](*concat_in, *concat_zeros)
        _dbg("run_spmd: dispatched (async)")
        for a in out_arrs:
            for sh in a.addressable_shards:
                sh.data.copy_to_host_async()
        _dbg("run_spmd: copy_to_host_async issued")
        r = {
            name: np.asarray(out_arrs[i]).reshape(NCORES, *out_avals[i][0])
            for i, name in enumerate(out_names)
        }
        _dbg("run_spmd: outputs on host")
        return r

    try:
        res = _call()
    except Exception:
        # compile needed but hook absent (stale marker) -> install, redo
        _ensure_hook(cdir, force=True)
        _NC_CACHE["compiled"] = _compile_exec(cdir, *_NC_CACHE["prog"])
        res = _call()
    try:
        open(os.path.join(cdir, f"ok_{_prog_key()}"), "w").close()
    except Exception:
        pass
    return res


def kernel(**inputs):
    import os
    _dbg("kernel: called")
    cdir = os.environ.get("JAX_COMPILATION_CACHE_DIR", "/tmp/jaxcache")
    in_maps = _host_prep(inputs)
    _dbg("kernel: host_prep done")
    if _BG_THREAD is not None:
        _BG_THREAD.join()
    _dbg("kernel: bg thread joined")
    if "compiled" not in _NC_CACHE:
        _prepare(cdir)   # background warmup failed or absent: do it now
    outs = _run_spmd(in_maps, cdir)
    _dbg("kernel: run_spmd done")
    full = np.ascontiguousarray(
        outs["out"].reshape(NCORES, PLOC).reshape(P)[:, None])
    kernel._last_res = None
    return full


# The per-core global input shapes, in _get_program's in_names order,
# then the donated output buffers (kept in sync with _build_nc's tensors).
import ml_dtypes as _mld
_NC_CACHE["gavals"] = [
    ((64, PLOC), np.dtype(_mld.bfloat16)), ((64, (L + N) // 8), np.dtype('<f4')),
    ((16, 832), np.dtype('<f4')),
    ((16, (2 * T * PLOC + L * K + N * K2) // 16), np.dtype('<i2')),
    ((1, T * PLOC), np.dtype('<f4')), ((1, PLOC), np.dtype('<f4')),
]


def _bg_warmup():
    import os
    try:
        _prepare(os.environ.get("JAX_COMPILATION_CACHE_DIR", "/tmp/jaxcache"))
    except Exception:
        _NC_CACHE.pop("compiled", None)


_BG_THREAD = None
try:
    import threading
    _BG_THREAD = threading.Thread(target=_bg_warmup, daemon=True)
    _BG_THREAD.start()
except Exception:
    _BG_THREAD = None

